# revision 6
# baseline (speedup 1.0000x reference)
"""MoE layer (8 experts, top-2) on 8 TRN2 NeuronCores — expert parallelism.

Contract: kernel(**inputs) takes FULL inputs, returns FULL output.
Strategy:
  - Host computes the (tiny) gate: logits -> top-2 -> softmax. This is the
    dispatch step of expert parallelism: tokens are gathered per expert.
  - Core e gets expert e's weights and its gathered tokens (padded to a
    fixed capacity C), computes y = relu(x @ w1 + b1) @ w2 scaled by the
    gate prob, via a Bass/Tile kernel using float32r matmuls.
  - Host scatter-adds the two expert contributions per token (combine step)
    plus the (usually zero) b2 term.

Shapes (hardcoded from the problem spec):
  x [2048, 2, 1024], gate_w [1024, 8], gate_b [8],
  w1 [8, 1024, 4096], b1 [8, 4096], w2 [8, 4096, 1024], b2 [8, 1024].
"""
import sys
import numpy as np

for _p in ("/opt/trn_rl_repo", "/root/.axon_site/_ro/trn_rl_repo"):
    if _p not in sys.path:
        sys.path.insert(0, _p)

import concourse.bacc as bacc
import concourse.tile as tile
import concourse.mybir as mybir
from concourse import bass_utils

N_EXPERTS = 8
TOP_K = 2
S, B, D, F = 2048, 2, 1024, 4096
P = 128
FB = 512                # F-block size streamed through SBUF
NB = F // FB            # 8 F-blocks
FC = FB // P            # 4 partition-tiles of F per block
DK = D // P             # 8 contraction tiles for stage 1
DN = D // 512           # 2 output-column chunks for stage 2

_f32 = mybir.dt.float32
_f32r = mybir.dt.float32r

_NC_CACHE: dict = {}
LAST_DEVICE_NS = -1  # wall-clock of the last run_bass_kernel_spmd call (incl. dispatch)


def _c_chunks(C):
    """Split C into chunks <=512 and >=256 (f32r full-rate needs N>=256)."""
    out = []
    pos, rem = 0, C
    while rem > 0:
        if rem >= 768:
            s = 512
        elif rem > 512:
            s = 256
        else:
            s = rem
        out.append((pos, s))
        pos += s
        rem -= s
    return out


def _build(C):
    """Trace + compile the per-core SPMD program for capacity C (multiple of 128, >=256)."""
    if C in _NC_CACHE:
        return _NC_CACHE[C]
    TT = C // P
    nc = bacc.Bacc("TRN2", target_bir_lowering=False, debug=False,
                   enable_asserts=False, num_devices=8)
    xgt_d = nc.dram_tensor("xgt", (D, C), _f32r, kind="ExternalInput").ap()
    w1_d = nc.dram_tensor("w1", (D, F), _f32r, kind="ExternalInput").ap()
    b1_d = nc.dram_tensor("b1", (F,), _f32, kind="ExternalInput").ap()
    w2_d = nc.dram_tensor("w2", (F, D), _f32r, kind="ExternalInput").ap()
    probs_d = nc.dram_tensor("probs", (C,), _f32, kind="ExternalInput").ap()
    y_d = nc.dram_tensor("y", (C, D), _f32, kind="ExternalOutput").ap()

    xgt_r = xgt_d.rearrange("(ko ki) c -> ki ko c", ki=P)      # [128, 8, C]
    w1_r = w1_d.rearrange("(ko ki) f -> ki ko f", ki=P)        # [128, 8, F]
    w2_r = w2_d.rearrange("(fo fi) d -> fi fo d", fi=P)        # [128, 32, D]
    y_r = y_d.rearrange("(t p) d -> p t d", p=P)               # [128, TT, D]

    chunks = _c_chunks(C)

    with tile.TileContext(nc) as tc:
        with tc.tile_pool(name="const", bufs=1) as cpool, \
             tc.tile_pool(name="w1p", bufs=2) as w1pool, \
             tc.tile_pool(name="w2p", bufs=2) as w2pool, \
             tc.tile_pool(name="hp", bufs=2) as hpool, \
             tc.tile_pool(name="ps1", bufs=3, space="PSUM") as psum1, \
             tc.tile_pool(name="ps2", bufs=3, space="PSUM") as psum2:

            xgt_sb = cpool.tile([P, DK, C], _f32r)
            for dk in range(DK):
                nc.sync.dma_start(xgt_sb[:, dk], xgt_r[:, dk])
            b1_sb = cpool.tile([P, F // P], _f32)
            nc.sync.dma_start(b1_sb[:], b1_d.rearrange("(o p) -> p o", p=P))
            probs_sb = cpool.tile([P, TT], _f32)
            nc.sync.dma_start(probs_sb[:], probs_d.rearrange("(o p) -> p o", p=P))
            y_acc = cpool.tile([P, TT, D], _f32)

            for fb in range(NB):
                w1_t = w1pool.tile([P, DK, FB], _f32r)
                for dk in range(DK):
                    nc.sync.dma_start(w1_t[:, dk], w1_r[:, dk, fb * FB:(fb + 1) * FB])
                w2_t = w2pool.tile([P, FC, D], _f32r)
                for fk in range(FC):
                    nc.sync.dma_start(w2_t[:, fk], w2_r[:, fb * FC + fk, :])

                # stage 1: hT[f, c] = relu(w1.T @ x + b1) for this F block
                hT = hpool.tile([P, FC, C], _f32r)
                for fc in range(FC):
                    for (cs, csz) in chunks:
                        ps = psum1.tile([P, 512], _f32)
                        for dk in range(DK):
                            nc.tensor.matmul(
                                ps[:, :csz],
                                w1_t[:, dk, fc * P:(fc + 1) * P],
                                xgt_sb[:, dk, cs:cs + csz],
                                start=(dk == 0), stop=(dk == DK - 1),
                            )
                        fcol = fb * FC + fc
                        nc.scalar.activation(
                            hT[:, fc, cs:cs + csz], ps[:, :csz],
                            mybir.ActivationFunctionType.Relu,
                            bias=b1_sb[:, fcol:fcol + 1], scale=1.0,
                        )

                # stage 2: y_acc[t, d] += hT.T @ w2 for this F block
                for tc_i in range(TT):
                    for dn in range(DN):
                        ps2 = psum2.tile([P, 512], _f32)
                        for fk in range(FC):
                            nc.tensor.matmul(
                                ps2[:],
                                hT[:, fk, tc_i * P:(tc_i + 1) * P],
                                w2_t[:, fk, dn * 512:(dn + 1) * 512],
                                start=(fk == 0), stop=(fk == FC - 1),
                            )
                        ya = y_acc[:, tc_i, dn * 512:(dn + 1) * 512]
                        if fb == 0:
                            nc.vector.tensor_copy(ya, ps2[:])
                        else:
                            nc.vector.tensor_add(ya, ps2[:], ya)
                        if fb == NB - 1:
                            nc.scalar.activation(
                                ya, ya, mybir.ActivationFunctionType.Copy,
                                scale=probs_sb[:, tc_i:tc_i + 1],
                            )
                            nc.sync.dma_start(y_r[:, tc_i, dn * 512:(dn + 1) * 512], ya)
    nc.compile()
    _NC_CACHE[C] = nc
    return nc


def _route(x2d, gate_w, gate_b):
    """Host gate: returns per-token top-2 expert ids and softmax probs (fp32)."""
    logits = x2d.astype(np.float64) @ gate_w.astype(np.float64) + gate_b.astype(np.float64)
    order = np.argsort(-logits, axis=-1, kind="stable")
    top2 = order[:, :TOP_K]                               # [T, 2]
    l = np.take_along_axis(logits, top2, axis=-1)         # [T, 2]
    m = l.max(axis=-1, keepdims=True)
    e = np.exp(l - m)
    p = (e / e.sum(axis=-1, keepdims=True)).astype(np.float32)
    return top2, p


def kernel(x, gate_w, gate_b, w1, b1, w2, b2):
    x = np.asarray(x, dtype=np.float32)
    gate_w = np.asarray(gate_w, dtype=np.float32)
    gate_b = np.asarray(gate_b, dtype=np.float32)
    w1 = np.asarray(w1, dtype=np.float32)
    b1 = np.asarray(b1, dtype=np.float32)
    w2 = np.asarray(w2, dtype=np.float32)
    b2 = np.asarray(b2, dtype=np.float32)

    T = S * B
    x2d = np.ascontiguousarray(x.reshape(T, D))
    top2, p = _route(x2d, gate_w, gate_b)

    # dispatch: token lists per expert
    idx_lists = []
    for e in range(N_EXPERTS):
        sel = np.nonzero(top2 == e)          # (token_idx, slot_idx)
        idx_lists.append((sel[0], p[sel[0], sel[1]]))
    max_n = max(len(ix) for ix, _ in idx_lists)

    # capacity cap (SBUF budget): if wildly imbalanced, run multiple passes
    C_MAX = 1280
    n_pass = max(1, -(-max_n // C_MAX))
    per_pass = -(-max_n // n_pass)
    C = max(256, -(-per_pass // P) * P)

    nc = _build(C)

    out2d = np.zeros((T, D), dtype=np.float32)
    xT = x2d.T  # [D, T]
    for ps in range(n_pass):
        in_maps = []
        metas = []
        for e in range(N_EXPERTS):
            ix_all, pe_all = idx_lists[e]
            ix = ix_all[ps * C:(ps + 1) * C]
            pe = pe_all[ps * C:(ps + 1) * C]
            n = len(ix)
            xgt = np.zeros((D, C), dtype=np.float32)
            if n:
                xgt[:, :n] = xT[:, ix]
            probs = np.zeros((C,), dtype=np.float32)
            probs[:n] = pe
            in_maps.append({
                "xgt": xgt,
                "w1": np.ascontiguousarray(w1[e]),
                "b1": np.ascontiguousarray(b1[e]),
                "w2": np.ascontiguousarray(w2[e]),
                "probs": probs,
            })
            metas.append((ix, n))
        import time as _time
        _t0 = _time.time()
        res = bass_utils.run_bass_kernel_spmd(nc, in_maps, core_ids=list(range(N_EXPERTS)))
        global LAST_DEVICE_NS
        LAST_DEVICE_NS = int((_time.time() - _t0) * 1e9)
        for e in range(N_EXPERTS):
            ix, n = metas[e]
            if n:
                out2d[ix] += res.results[e]["y"][:n]  # ix unique per expert

    if np.any(b2):
        comb = np.zeros((T, N_EXPERTS), dtype=np.float32)
        np.put_along_axis(comb, top2, p, axis=-1)
        out2d += comb @ b2
    return out2d.reshape(S, B, D)


# revision 14
# speedup vs baseline: 90.8861x; 90.8861x over previous
"""MoE layer (8 experts, top-2) on 8 TRN2 NeuronCores — expert parallelism.

Contract: kernel(**inputs) takes FULL inputs, returns FULL output.
Strategy:
  - Host computes the (tiny) gate: logits -> top-2 -> softmax. This is the
    dispatch step of expert parallelism: tokens are gathered per expert.
  - Core e gets expert e's weights and its gathered tokens (padded to a
    fixed capacity C), computes y = relu(x @ w1 + b1) @ w2 scaled by the
    gate prob, via a Bass/Tile kernel using float32r matmuls.
  - Host scatter-adds the two expert contributions per token (combine step)
    plus the (usually zero) b2 term.

Shapes (hardcoded from the problem spec):
  x [2048, 2, 1024], gate_w [1024, 8], gate_b [8],
  w1 [8, 1024, 4096], b1 [8, 4096], w2 [8, 4096, 1024], b2 [8, 1024].
"""
import sys
import numpy as np

for _p in ("/opt/trn_rl_repo", "/root/.axon_site/_ro/trn_rl_repo"):
    if _p not in sys.path:
        sys.path.insert(0, _p)

import concourse.bacc as bacc
import concourse.tile as tile
import concourse.mybir as mybir
from concourse import bass_utils, bass2jax, mybir as _mybir

N_EXPERTS = 8
TOP_K = 2
S, B, D, F = 2048, 2, 1024, 4096
P = 128
FB = 512                # F-block size streamed through SBUF
NB = F // FB            # 8 F-blocks
FC = FB // P            # 4 partition-tiles of F per block
DK = D // P             # 8 contraction tiles for stage 1
DN = D // 512           # 2 output-column chunks for stage 2

_f32 = mybir.dt.float32
_f32r = mybir.dt.float32r

_NC_CACHE: dict = {}
LAST_DEVICE_NS = -1  # wall-clock of the last device dispatch (incl. transfers)
LAST_C = -1


def _c_chunks(C):
    """Split C into chunks <=512 and >=256 (f32r full-rate needs N>=256)."""
    out = []
    pos, rem = 0, C
    while rem > 0:
        if rem >= 768:
            s = 512
        elif rem > 512:
            s = 256
        else:
            s = rem
        out.append((pos, s))
        pos += s
        rem -= s
    return out


def _build(C):
    """Trace + compile the per-core SPMD program for capacity C (multiple of 128, >=256)."""
    if C in _NC_CACHE:
        return _NC_CACHE[C]
    TT = C // P
    nc = bacc.Bacc("TRN2", target_bir_lowering=False, debug=False,
                   enable_asserts=False, num_devices=8)
    xgt_d = nc.dram_tensor("xgt", (D, C), _f32r, kind="ExternalInput").ap()
    w1_d = nc.dram_tensor("w1", (D, F), _f32r, kind="ExternalInput").ap()
    b1_d = nc.dram_tensor("b1", (F,), _f32, kind="ExternalInput").ap()
    w2_d = nc.dram_tensor("w2", (F, D), _f32r, kind="ExternalInput").ap()
    probs_d = nc.dram_tensor("probs", (C,), _f32, kind="ExternalInput").ap()
    y_d = nc.dram_tensor("y", (C, D), _f32, kind="ExternalOutput").ap()

    xgt_r = xgt_d.rearrange("(ko ki) c -> ki ko c", ki=P)      # [128, 8, C]
    w1_r = w1_d.rearrange("(ko ki) f -> ki ko f", ki=P)        # [128, 8, F]
    w2_r = w2_d.rearrange("(fo fi) d -> fi fo d", fi=P)        # [128, 32, D]
    y_r = y_d.rearrange("(t p) d -> p t d", p=P)               # [128, TT, D]

    chunks = _c_chunks(C)

    with tile.TileContext(nc) as tc:
        with tc.tile_pool(name="const", bufs=1) as cpool, \
             tc.tile_pool(name="w1p", bufs=2) as w1pool, \
             tc.tile_pool(name="w2p", bufs=2) as w2pool, \
             tc.tile_pool(name="hp", bufs=2) as hpool, \
             tc.tile_pool(name="ps1", bufs=3, space="PSUM") as psum1, \
             tc.tile_pool(name="ps2", bufs=3, space="PSUM") as psum2:

            xgt_sb = cpool.tile([P, DK, C], _f32r)
            for dk in range(DK):
                nc.sync.dma_start(xgt_sb[:, dk], xgt_r[:, dk])
            b1_sb = cpool.tile([P, F // P], _f32)
            nc.sync.dma_start(b1_sb[:], b1_d.rearrange("(o p) -> p o", p=P))
            probs_sb = cpool.tile([P, TT], _f32)
            nc.sync.dma_start(probs_sb[:], probs_d.rearrange("(o p) -> p o", p=P))
            y_acc = cpool.tile([P, TT, D], _f32)

            for fb in range(NB):
                w1_t = w1pool.tile([P, DK, FB], _f32r)
                for dk in range(DK):
                    nc.sync.dma_start(w1_t[:, dk], w1_r[:, dk, fb * FB:(fb + 1) * FB])
                w2_t = w2pool.tile([P, FC, D], _f32r)
                for fk in range(FC):
                    nc.sync.dma_start(w2_t[:, fk], w2_r[:, fb * FC + fk, :])

                # stage 1: hT[f, c] = relu(w1.T @ x + b1) for this F block
                hT = hpool.tile([P, FC, C], _f32r)
                for fc in range(FC):
                    for (cs, csz) in chunks:
                        ps = psum1.tile([P, 512], _f32)
                        for dk in range(DK):
                            nc.tensor.matmul(
                                ps[:, :csz],
                                w1_t[:, dk, fc * P:(fc + 1) * P],
                                xgt_sb[:, dk, cs:cs + csz],
                                start=(dk == 0), stop=(dk == DK - 1),
                            )
                        fcol = fb * FC + fc
                        nc.scalar.activation(
                            hT[:, fc, cs:cs + csz], ps[:, :csz],
                            mybir.ActivationFunctionType.Relu,
                            bias=b1_sb[:, fcol:fcol + 1], scale=1.0,
                        )

                # stage 2: y_acc[t, d] += hT.T @ w2 for this F block
                for tc_i in range(TT):
                    for dn in range(DN):
                        ps2 = psum2.tile([P, 512], _f32)
                        for fk in range(FC):
                            nc.tensor.matmul(
                                ps2[:],
                                hT[:, fk, tc_i * P:(tc_i + 1) * P],
                                w2_t[:, fk, dn * 512:(dn + 1) * 512],
                                start=(fk == 0), stop=(fk == FC - 1),
                            )
                        ya = y_acc[:, tc_i, dn * 512:(dn + 1) * 512]
                        if fb == 0:
                            nc.vector.tensor_copy(ya, ps2[:])
                        else:
                            nc.vector.tensor_add(ya, ps2[:], ya)
                        if fb == NB - 1:
                            nc.scalar.activation(
                                ya, ya, mybir.ActivationFunctionType.Copy,
                                scale=probs_sb[:, tc_i:tc_i + 1],
                            )
                            nc.sync.dma_start(y_r[:, tc_i, dn * 512:(dn + 1) * 512], ya)
    nc.compile()
    _NC_CACHE[C] = nc
    return nc


class _Runner:
    """Persistent jitted SPMD executor for a compiled Bacc program.

    Mirrors bass2jax.run_bass_via_pjrt but keeps the jitted callable so
    repeat calls skip retracing/recompiling.
    """

    def __init__(self, nc, n_cores):
        import jax
        from jax.sharding import Mesh, PartitionSpec
        from jax.experimental.shard_map import shard_map

        bass2jax.install_neuronx_cc_hook()
        self.nc = nc
        self.n_cores = n_cores
        in_names, out_names, out_avals = [], [], []
        for alloc in nc.m.functions[0].allocations:
            if not isinstance(alloc, _mybir.MemoryLocationSet):
                continue
            name = alloc.memorylocations[0].name
            if alloc.kind == "ExternalInput":
                in_names.append(name)
            elif alloc.kind == "ExternalOutput":
                out_names.append(name)
                out_avals.append(jax.core.ShapedArray(
                    tuple(alloc.tensor_shape), _mybir.dt.np(alloc.dtype)))
        partition_name = nc.partition_id_tensor.name if nc.partition_id_tensor else None
        in_names = [n for n in in_names if n != partition_name]
        all_names = in_names + out_names + ([partition_name] if partition_name else [])
        self.in_names, self.out_names, self.out_avals = in_names, out_names, out_avals
        n_params = len(in_names)

        def _body(*args):
            operands = list(args)
            if partition_name is not None:
                operands.append(bass2jax.partition_id_tensor())
            outs = bass2jax._bass_exec_p.bind(
                *operands,
                out_avals=tuple(out_avals),
                in_names=tuple(all_names),
                out_names=tuple(out_names),
                lowering_input_output_aliases=(),
                sim_require_finite=False,
                sim_require_nnan=False,
                nc=nc,
            )
            return tuple(outs)

        devices = jax.devices()[:n_cores]
        mesh = Mesh(np.asarray(devices), ("core",))
        n_outs = len(out_names)
        self._fn = jax.jit(
            shard_map(_body, mesh=mesh,
                      in_specs=(PartitionSpec("core"),) * (n_params + n_outs),
                      out_specs=(PartitionSpec("core"),) * n_outs,
                      check_rep=False),
            donate_argnums=tuple(range(n_params, n_params + n_outs)),
            keep_unused=True,
        )
        self._jax = jax

    def concat_inputs(self, in_maps):
        return [np.concatenate([np.asarray(m[name]) for m in in_maps], axis=0)
                for name in self.in_names]

    def zero_outs(self):
        jnp = self._jax.numpy
        return [jnp.zeros((self.n_cores * a.shape[0], *a.shape[1:]), a.dtype)
                for a in self.out_avals]

    def run_raw(self, concat_in, zouts):
        outs = self._fn(*concat_in, *zouts)
        self._jax.block_until_ready(outs)
        return outs

    def run(self, in_maps):
        outs = self.run_raw(self.concat_inputs(in_maps), self.zero_outs())
        return [
            {name: np.asarray(outs[i]).reshape(self.n_cores, *self.out_avals[i].shape)[c]
             for i, name in enumerate(self.out_names)}
            for c in range(self.n_cores)
        ]


_RUNNER_CACHE: dict = {}


def _runner(C):
    if C not in _RUNNER_CACHE:
        _RUNNER_CACHE[C] = _Runner(_build(C), N_EXPERTS)
    return _RUNNER_CACHE[C]


def _route(x2d, gate_w, gate_b):
    """Host gate: returns per-token top-2 expert ids and softmax probs (fp32)."""
    logits = x2d.astype(np.float64) @ gate_w.astype(np.float64) + gate_b.astype(np.float64)
    order = np.argsort(-logits, axis=-1, kind="stable")
    top2 = order[:, :TOP_K]                               # [T, 2]
    l = np.take_along_axis(logits, top2, axis=-1)         # [T, 2]
    m = l.max(axis=-1, keepdims=True)
    e = np.exp(l - m)
    p = (e / e.sum(axis=-1, keepdims=True)).astype(np.float32)
    return top2, p


def kernel(x, gate_w, gate_b, w1, b1, w2, b2):
    x = np.asarray(x, dtype=np.float32)
    gate_w = np.asarray(gate_w, dtype=np.float32)
    gate_b = np.asarray(gate_b, dtype=np.float32)
    w1 = np.asarray(w1, dtype=np.float32)
    b1 = np.asarray(b1, dtype=np.float32)
    w2 = np.asarray(w2, dtype=np.float32)
    b2 = np.asarray(b2, dtype=np.float32)

    T = S * B
    x2d = np.ascontiguousarray(x.reshape(T, D))
    top2, p = _route(x2d, gate_w, gate_b)

    # dispatch: token lists per expert
    idx_lists = []
    for e in range(N_EXPERTS):
        sel = np.nonzero(top2 == e)          # (token_idx, slot_idx)
        idx_lists.append((sel[0], p[sel[0], sel[1]]))
    max_n = max(len(ix) for ix, _ in idx_lists)

    # capacity cap (SBUF budget): if wildly imbalanced, run multiple passes
    C_MAX = 1280
    n_pass = max(1, -(-max_n // C_MAX))
    per_pass = -(-max_n // n_pass)
    C = max(256, -(-per_pass // P) * P)

    global LAST_C
    LAST_C = C
    runner = _runner(C)

    out2d = np.zeros((T, D), dtype=np.float32)
    xT = x2d.T  # [D, T]
    for ps in range(n_pass):
        in_maps = []
        metas = []
        for e in range(N_EXPERTS):
            ix_all, pe_all = idx_lists[e]
            ix = ix_all[ps * C:(ps + 1) * C]
            pe = pe_all[ps * C:(ps + 1) * C]
            n = len(ix)
            xgt = np.zeros((D, C), dtype=np.float32)
            if n:
                xgt[:, :n] = xT[:, ix]
            probs = np.zeros((C,), dtype=np.float32)
            probs[:n] = pe
            in_maps.append({
                "xgt": xgt,
                "w1": np.ascontiguousarray(w1[e]),
                "b1": np.ascontiguousarray(b1[e]),
                "w2": np.ascontiguousarray(w2[e]),
                "probs": probs,
            })
            metas.append((ix, n))
        import time as _time
        _t0 = _time.time()
        results = runner.run(in_maps)
        global LAST_DEVICE_NS
        LAST_DEVICE_NS = int((_time.time() - _t0) * 1e9)
        for e in range(N_EXPERTS):
            ix, n = metas[e]
            if n:
                out2d[ix] += results[e]["y"][:n]  # ix unique per expert

    if np.any(b2):
        comb = np.zeros((T, N_EXPERTS), dtype=np.float32)
        np.put_along_axis(comb, top2, p, axis=-1)
        out2d += comb @ b2
    return out2d.reshape(S, B, D)
